# revision 9
# baseline (speedup 1.0000x reference)
"""Trainium2 Bass kernel for nn_AttentionBlock (B=4, S=2048, E=512, H=8).

Sharding (no cross-core communication):
  core c owns batch b = c//2 and output rows for tokens half = c%2
  (1024 tokens).  Each core computes Q for its own 1024 tokens and K/V for
  the full batch (2048 tokens); K/V projections are duplicated between the
  two cores of a batch (cheap) so attention and the output projection are
  fully local.

Host-side layout prep (inside kernel(), numpy only):
  - x[b] is permuted so the core's own 1024 query tokens are rows 0:1024,
    then transposed to feature-major xkv_t [512, 2048].  Softmax/PV are
    permutation-invariant in the key axis, so K/V token order is irrelevant.
  - weights are passed as W.T [E, E] so lhsT/rhs tiles are direct DMA loads.

Device dataflow (per core, all f32):
  A) QT[f, t] = gelu(Wq @ x^T + bq) via PE (lhsT = Wq.T chunk, rhs = x^T),
     same for KT; V natural [t, f] via (lhsT = x^T chunk, rhs = Wv.T chunk)
     with the bias row pre-seeded into PSUM by a K=1 ones matmul.
  B) per head pair (even head at partitions 0:64, odd at 64:128 so the
     K=64 S^T matmuls auto-pack into disjoint PE row groups):
     S^T tile = K^T_h(ktile) @ Q^T_h -> PSUM, DVE-copied to a big SBUF
     stage, one big ACT Exp(scale=1/8) in place, then PV accumulates
     O^T_unnorm and sumexp in one matmul via a [V | 1] lhsT (65 cols).
  C) per-head normalization deferred to one pass: sumexp rows gathered
     into SE tiles, reciprocal on DVE, broadcast across 64 partitions with
     a K=2 selector matmul, one DVE multiply per ORT chunk.
  D) out = gelu(ORT chunks @ Wo.T + bo) with bo PSUM-seeded; DMA out.
"""

import numpy as np

import concourse.bass as bass
import concourse.tile as tile
from concourse import bacc, mybir

F32 = mybir.dt.float32
AF = mybir.ActivationFunctionType

E = 512          # embed dim
H = 8            # heads
D = 64           # head dim
P = 128          # partitions
EC = E // P      # 128-row chunks of the embed dim
B = 4
S = 2048
N_CORES = 8
SCALE = 0.125    # 1/sqrt(D)


def build(tc, io, T_KV, T_Q, stop_after="all"):
    """Emit the per-core program.  T_KV: key/value tokens; T_Q: query tokens
    (the first T_Q columns of xkv_t).  stop_after: debug knob
    ("setup"|"proj"|"st"|"pv"|"norm"|"all")."""
    nc = tc.nc
    stages = ["setup", "proj", "st", "pv", "norm", "all"]
    lvl = stages.index(stop_after)
    n_g = T_KV // 512        # 512-token groups for projections
    n_qg = T_Q // 512        # q groups (S^T rhs is N=512)
    n_kt = T_KV // P         # 128-token key tiles
    SG = min(8, n_kt)        # ktiles per exp stage
    n_sg = n_kt // SG
    assert T_KV % 512 == 0 and T_Q % 512 == 0 and n_kt % SG == 0 and SG % 2 == 0

    xkv = io["xkv_t"]
    out = io["out"]

    with tc.tile_pool(name="persist", bufs=1) as persist, \
         tc.tile_pool(name="ps", space="PSUM", bufs=1) as psp:
        # ---------------- persistent SBUF state ----------------
        wo_sb = [persist.tile([P, E], F32, name=f"wo_sb{k}", tag=f"wo{k}")
                 for k in range(EC)]
        QT = [persist.tile([P, T_Q], F32, name=f"qt_sb{m}", tag=f"qt{m}")
              for m in range(EC)]
        KT = [persist.tile([P, T_KV], F32, name=f"kt_sb{m}", tag=f"kt{m}")
              for m in range(EC)]
        V3 = [persist.tile([P, H, 66], F32, name=f"v3_sb{t}", tag=f"v3{t}")
              for t in range(n_kt)]
        ORT = [persist.tile([P, T_Q], F32, name=f"ort_sb{m}", tag=f"ort{m}")
               for m in range(EC)]
        # sumexp rows live at partition 64 (even head) and 32 (odd head):
        # SBUF accesses may only start at partitions {0, 32, 64, 96} and
        # matmul operand base partitions only at {0, 32, 64}.
        SE = [persist.tile([65, T_Q], F32, name=f"se_sb{hp}", tag=f"se{hp}")
              for hp in range(H // 2)]
        ones1 = persist.tile([1, P], F32, name="ones1")
        sel2 = persist.tile([65, P], F32, name="sel2")
        bv_row = persist.tile([1, E], F32, name="bv_row")
        bo_row = persist.tile([1, E], F32, name="bo_row")

        nc.vector.memset(ones1, 1.0)
        # sel2 rows 64 / 32 are 64-wide ones rows used as K=1 lhsT to
        # broadcast the even / odd head's reciprocal sumexp across 64
        # output partitions.  (The two broadcasts must stay separate
        # single-instruction accumulation groups: mixing lhsT base
        # partitions inside one start/stop group crashes the HW.)
        nc.vector.memset(sel2[64:65, 0:64], 1.0)
        nc.vector.memset(sel2[32:33, 0:64], 1.0)
        for t in range(n_kt):
            nc.gpsimd.memset(V3[t][:, :, 64:66], 1.0)
        for k in range(EC):
            nc.sync.dma_start(out=wo_sb[k], in_=io["wo_t"][k * P:(k + 1) * P, :])
        nc.sync.dma_start(out=bv_row, in_=io["bv_row"])
        nc.sync.dma_start(out=bo_row, in_=io["bo_row"])

        # ---------------- phase A: projections ----------------
        with tc.tile_pool(name="projw", bufs=1) as projw, \
             tc.tile_pool(name="xtp", bufs=2) as xtp:
            wq_sb = [projw.tile([P, E], F32, name=f"wq_sb{k}", tag=f"wq{k}")
                     for k in range(EC)]
            wk_sb = [projw.tile([P, E], F32, name=f"wk_sb{k}", tag=f"wk{k}")
                     for k in range(EC)]
            wv_sb = [projw.tile([P, E], F32, name=f"wv_sb{k}", tag=f"wv{k}")
                     for k in range(EC)]
            bq_sb = [projw.tile([P, 1], F32, name=f"bq_sb{m}", tag=f"bq{m}")
                     for m in range(EC)]
            bk_sb = [projw.tile([P, 1], F32, name=f"bk_sb{m}", tag=f"bk{m}")
                     for m in range(EC)]
            for k in range(EC):
                ksl = slice(k * P, (k + 1) * P)
                nc.sync.dma_start(out=wq_sb[k], in_=io["wq_t"][ksl, :])
                nc.sync.dma_start(out=wk_sb[k], in_=io["wk_t"][ksl, :])
                nc.sync.dma_start(out=wv_sb[k], in_=io["wv_t"][ksl, :])
                nc.sync.dma_start(out=bq_sb[k], in_=io["bq_col"][ksl, :])
                nc.sync.dma_start(out=bk_sb[k], in_=io["bk_col"][ksl, :])

            for g in range(n_g if lvl >= 1 else 0):
                gsl = slice(g * 512, (g + 1) * 512)
                xt = [xtp.tile([P, 512], F32, name=f"xt_g{g}k{k}", tag=f"xt{k}")
                      for k in range(EC)]
                for k in range(EC):
                    nc.sync.dma_start(out=xt[k], in_=xkv[k * P:(k + 1) * P, gsl])
                # K^T chunks
                for m in range(EC):
                    msl = slice(m * P, (m + 1) * P)
                    ps = psp.tile([P, 512], F32, name=f"ps_k{g}_{m}",
                                  tag="st", bufs=3)
                    for k in range(EC):
                        nc.tensor.matmul(ps, lhsT=wk_sb[k][:, msl], rhs=xt[k],
                                         start=(k == 0), stop=(k == EC - 1))
                    nc.scalar.activation(KT[m][:, gsl], ps, AF.Gelu,
                                         bias=bk_sb[m])
                # Q^T chunks (only the core's own tokens)
                if g < n_qg:
                    for m in range(EC):
                        msl = slice(m * P, (m + 1) * P)
                        ps = psp.tile([P, 512], F32, name=f"ps_q{g}_{m}",
                                      tag="st", bufs=3)
                        for k in range(EC):
                            nc.tensor.matmul(ps, lhsT=wq_sb[k][:, msl],
                                             rhs=xt[k],
                                             start=(k == 0), stop=(k == EC - 1))
                        nc.scalar.activation(QT[m][:, gsl], ps, AF.Gelu,
                                             bias=bq_sb[m])
                # V natural [token, feature], bias seeded via K=1 matmul
                for s4 in range(4):
                    t = g * 4 + s4
                    ps = psp.tile([P, E], F32, name=f"ps_v{t}", tag="st",
                                  bufs=3)
                    nc.tensor.matmul(ps, lhsT=ones1, rhs=bv_row,
                                     start=True, stop=False)
                    for k in range(EC):
                        nc.tensor.matmul(ps,
                                         lhsT=xt[k][:, s4 * P:(s4 + 1) * P],
                                         rhs=wv_sb[k],
                                         start=False, stop=(k == EC - 1))
                    nc.scalar.activation(V3[t][:, :, 0:64],
                                         ps.rearrange("p (h d) -> p h d", h=H),
                                         AF.Gelu)

        # ---------------- phase B: attention ----------------
        with tc.tile_pool(name="stgp", bufs=3) as stgp, \
             tc.tile_pool(name="tmpp", bufs=2) as tmpp, \
             tc.tile_pool(name="outp", bufs=3) as outp:
            for hp in range(H // 2 if lvl >= 2 else 0):
                he, ho = 2 * hp, 2 * hp + 1
                for q in range(n_qg):
                    qsl = slice(q * 512, (q + 1) * 512)
                    pv0 = psp.tile([65, 512], F32, name=f"pv0_{hp}_{q}",
                                   tag="pv", bufs=2)
                    pv1 = psp.tile([65, 512], F32, name=f"pv1_{hp}_{q}",
                                   tag="pv", bufs=2)
                    for sg in range(n_sg):
                        stg0 = stgp.tile([P, SG * 512], F32,
                                         name=f"stg0_{hp}{q}{sg}", tag="stg")
                        stg1 = stgp.tile([P, SG * 512], F32,
                                         name=f"stg1_{hp}{q}{sg}", tag="stg")
                        for jp in range(SG // 2):
                            st0 = psp.tile([P, 1024], F32,
                                           name=f"st0_{hp}{q}{sg}{jp}",
                                           tag="st", bufs=3)
                            st1 = psp.tile([P, 1024], F32,
                                           name=f"st1_{hp}{q}{sg}{jp}",
                                           tag="st", bufs=3)
                            for u in range(2):
                                kt = sg * SG + jp * 2 + u
                                ksl = slice(kt * P, (kt + 1) * P)
                                usl = slice(u * 512, (u + 1) * 512)
                                nc.tensor.matmul(st0[:, usl],
                                                 lhsT=KT[hp][0:64, ksl],
                                                 rhs=QT[hp][0:64, qsl],
                                                 start=True, stop=True)
                                nc.tensor.matmul(st1[:, usl],
                                                 lhsT=KT[hp][64:128, ksl],
                                                 rhs=QT[hp][64:128, qsl],
                                                 start=True, stop=True)
                            jsl2 = slice(jp * 1024, (jp + 1) * 1024)
                            nc.vector.tensor_copy(stg0[:, jsl2], st0)
                            nc.vector.tensor_copy(stg1[:, jsl2], st1)
                        nc.scalar.activation(stg0, stg0, AF.Exp, scale=SCALE)
                        nc.scalar.activation(stg1, stg1, AF.Exp, scale=SCALE)
                        for j in range(SG if lvl >= 3 else 0):
                            kt = sg * SG + j
                            jsl = slice(j * 512, (j + 1) * 512)
                            nc.tensor.matmul(pv0, lhsT=V3[kt][:, he, 0:65],
                                             rhs=stg0[:, jsl],
                                             start=(kt == 0),
                                             stop=(kt == n_kt - 1))
                            nc.tensor.matmul(pv1, lhsT=V3[kt][:, ho, 0:65],
                                             rhs=stg1[:, jsl],
                                             start=(kt == 0),
                                             stop=(kt == n_kt - 1))
                    if lvl < 3:
                        continue
                    # unit drain: even head rows are partition-aligned, the
                    # odd head's hop across partitions goes via SBUF + DMA.
                    nc.vector.tensor_copy(ORT[hp][0:64, qsl], pv0[0:64, :])
                    nc.vector.tensor_copy(SE[hp][64:65, qsl], pv0[64:65, :])
                    tmp = tmpp.tile([65, 512], F32, name=f"tmp_{hp}_{q}",
                                    tag="tmp")
                    nc.vector.tensor_copy(tmp, pv1)
                    nc.sync.dma_start(out=ORT[hp][64:128, qsl],
                                      in_=tmp[0:64, :])
                    nc.sync.dma_start(out=SE[hp][32:33, qsl],
                                      in_=tmp[64:65, :])

            # ---------------- normalization ----------------
            for hp in range(H // 2 if lvl >= 4 else 0):
                nc.vector.reciprocal(SE[hp][64:65, :], SE[hp][64:65, :])
                nc.vector.reciprocal(SE[hp][32:33, :], SE[hp][32:33, :])
                R = psp.tile([P, T_Q], F32, name=f"R_{hp}", tag="st", bufs=3)
                for q in range(n_qg):
                    qsl = slice(q * 512, (q + 1) * 512)
                    nc.tensor.matmul(R[0:64, qsl], lhsT=sel2[64:65, 0:64],
                                     rhs=SE[hp][64:65, qsl],
                                     start=True, stop=True)
                    nc.tensor.matmul(R[64:128, qsl], lhsT=sel2[32:33, 0:64],
                                     rhs=SE[hp][32:33, qsl],
                                     start=True, stop=True)
                nc.vector.tensor_mul(ORT[hp], ORT[hp], R)

            # ---------------- phase C: output projection ----------------
            for t in range(T_Q // P if lvl >= 5 else 0):
                tsl = slice(t * P, (t + 1) * P)
                ps = psp.tile([P, E], F32, name=f"ps_o{t}", tag="st", bufs=3)
                nc.tensor.matmul(ps, lhsT=ones1, rhs=bo_row,
                                 start=True, stop=False)
                for m in range(EC):
                    nc.tensor.matmul(ps, lhsT=ORT[m][:, tsl], rhs=wo_sb[m],
                                     start=False, stop=(m == EC - 1))
                ot = outp.tile([P, E], F32, name=f"ot_{t}", tag="ot")
                nc.scalar.activation(ot, ps, AF.Gelu)
                nc.sync.dma_start(out=out[tsl, :], in_=ot)


def make_nc(T_KV, T_Q, num_devices=N_CORES, debug=False):
    nc = bacc.Bacc("TRN2", target_bir_lowering=False, debug=debug,
                   num_devices=num_devices)
    io = {
        "xkv_t": nc.dram_tensor("xkv_t", [E, T_KV], F32,
                                kind="ExternalInput").ap(),
        "wq_t": nc.dram_tensor("wq_t", [E, E], F32, kind="ExternalInput").ap(),
        "wk_t": nc.dram_tensor("wk_t", [E, E], F32, kind="ExternalInput").ap(),
        "wv_t": nc.dram_tensor("wv_t", [E, E], F32, kind="ExternalInput").ap(),
        "wo_t": nc.dram_tensor("wo_t", [E, E], F32, kind="ExternalInput").ap(),
        "bq_col": nc.dram_tensor("bq_col", [E, 1], F32,
                                 kind="ExternalInput").ap(),
        "bk_col": nc.dram_tensor("bk_col", [E, 1], F32,
                                 kind="ExternalInput").ap(),
        "bv_row": nc.dram_tensor("bv_row", [1, E], F32,
                                 kind="ExternalInput").ap(),
        "bo_row": nc.dram_tensor("bo_row", [1, E], F32,
                                 kind="ExternalInput").ap(),
        "out": nc.dram_tensor("out", [T_Q, E], F32, kind="ExternalOutput").ap(),
    }
    with tile.TileContext(nc) as tc:
        build(tc, io, T_KV=T_KV, T_Q=T_Q)
    nc.compile()
    return nc


def make_in_maps(x, Wq, bq, Wk, bk, Wv, bv, Wo, bo):
    cast = lambda a: np.ascontiguousarray(np.asarray(a, dtype=np.float32))
    base = {
        "wq_t": cast(np.asarray(Wq).T),
        "wk_t": cast(np.asarray(Wk).T),
        "wv_t": cast(np.asarray(Wv).T),
        "wo_t": cast(np.asarray(Wo).T),
        "bq_col": cast(np.asarray(bq)[:, None]),
        "bk_col": cast(np.asarray(bk)[:, None]),
        "bv_row": cast(np.asarray(bv)[None, :]),
        "bo_row": cast(np.asarray(bo)[None, :]),
    }
    x = np.asarray(x)
    half_len = S // 2
    in_maps = []
    for c in range(N_CORES):
        b, half = divmod(c, 2)
        xb = x[b]
        mine = xb[half * half_len:(half + 1) * half_len]
        oth = xb[(1 - half) * half_len:(2 - half) * half_len]
        m = dict(base)
        m["xkv_t"] = cast(np.concatenate([mine, oth], axis=0).T)
        in_maps.append(m)
    return in_maps


_NC_CACHE = {}


def _get_full_nc():
    if "full" not in _NC_CACHE:
        _NC_CACHE["full"] = make_nc(T_KV=S, T_Q=S // 2)
    return _NC_CACHE["full"]


def run_on_hw(in_maps, trace=False, **kw):
    from concourse.bass_utils import run_bass_kernel_spmd
    nc = _get_full_nc()
    return run_bass_kernel_spmd(nc, in_maps, core_ids=list(range(N_CORES)),
                                trace=trace, **kw)


def kernel(x, Wq, bq, Wk, bk, Wv, bv, Wo, bo):
    in_maps = make_in_maps(x, Wq, bq, Wk, bk, Wv, bv, Wo, bo)
    res = run_on_hw(in_maps)
    half_len = S // 2
    out = np.empty((B, S, E), np.float32)
    for c in range(N_CORES):
        b, half = divmod(c, 2)
        out[b, half * half_len:(half + 1) * half_len, :] = \
            res.results[c]["out"]
    return out


# revision 13
# speedup vs baseline: 2.7710x; 2.7710x over previous
"""Trainium2 Bass kernel for nn_AttentionBlock (B=4, S=2048, E=512, H=8).

Sharding (no cross-core communication):
  core c owns batch b = c//2 and output rows for tokens half = c%2
  (1024 tokens).  Each core computes Q for its own 1024 tokens and K/V for
  the full batch (2048 tokens); K/V projections are duplicated between the
  two cores of a batch (cheap) so attention and the output projection are
  fully local.

Host-side layout prep (inside kernel(), numpy only):
  - x[b] is permuted so the core's own 1024 query tokens are rows 0:1024,
    then transposed to feature-major xkv_t [512, 2048].  Softmax/PV are
    permutation-invariant in the key axis, so K/V token order is irrelevant.
  - weights are passed as W.T [E, E] so lhsT/rhs tiles are direct DMA loads.
  - x and weights are cast to bf16 (fp32 matmuls lower to two PE passes on
    TRN2 — half throughput); biases, PSUM accumulation, softmax statistics
    and the normalization stay fp32.

Device dataflow (per core):
  A) QT[f, t] = gelu(Wq @ x^T + bq) via PE (lhsT = Wq.T chunk, rhs = x^T),
     same for KT; V natural [t, f] via (lhsT = x^T chunk, rhs = Wv.T chunk)
     with the bias row pre-seeded into PSUM by a K=1 fp32 ones matmul.
  B) per head pair (even head at partitions 0:64, odd at 64:128 so the
     K=64 S^T matmuls auto-pack into disjoint PE row groups):
     S^T tile = K^T_h(ktile) @ Q^T_h -> PSUM (fp32), one ACT Exp(scale=1/8)
     per [128,1024] tile straight from PSUM into bf16 E tiles, then PV
     accumulates O^T_unnorm and sumexp in one matmul via a [V | 1] lhsT.
  C) per-head normalization deferred to one pass: sumexp rows at SBUF
     partitions 64 (even) / 32 (odd), fast-approx reciprocal on DVE,
     broadcast across 64 output partitions with single-instruction K=1
     selector matmuls (even -> R rows 0:64, odd -> R rows 64:128 — these
     must stay separate accumulation groups: mixing lhsT base partitions
     inside one start/stop group crashes the HW), one DVE multiply per
     ORT chunk.
  D) out = gelu(ORT chunks @ Wo.T + bo) with bo PSUM-seeded; DMA out.
"""

import numpy as np

import concourse.bass as bass
import concourse.tile as tile
from concourse import bacc, mybir

F32 = mybir.dt.float32
BF16 = mybir.dt.bfloat16
AF = mybir.ActivationFunctionType

E = 512          # embed dim
H = 8            # heads
D = 64           # head dim
P = 128          # partitions
EC = E // P      # 128-row chunks of the embed dim
B = 4
S = 2048
N_CORES = 8
SCALE = 0.125    # 1/sqrt(D)


def build(tc, io, T_KV, T_Q, mm_dt=BF16, stop_after="all"):
    """Emit the per-core program.  T_KV: key/value tokens; T_Q: query tokens
    (the first T_Q columns of xkv_t).  mm_dt: dtype of matmul operands."""
    nc = tc.nc
    stages = ["setup", "proj", "st", "pv", "norm", "all"]
    lvl = stages.index(stop_after)
    n_g = T_KV // 512        # 512-token groups for projections
    n_qg = T_Q // 512        # q groups (S^T rhs is N=512)
    n_kt = T_KV // P         # 128-token key tiles
    assert T_KV % 1024 == 0 and T_Q % 1024 == 0

    xkv = io["xkv_t"]
    out = io["out"]

    with tc.tile_pool(name="persist", bufs=1) as persist, \
         tc.tile_pool(name="ps", space="PSUM", bufs=1) as psp:
        # ---------------- persistent SBUF state ----------------
        wo_sb = [persist.tile([P, E], mm_dt, name=f"wo_sb{k}", tag=f"wo{k}")
                 for k in range(EC)]
        QT = [persist.tile([P, T_Q], mm_dt, name=f"qt_sb{m}", tag=f"qt{m}")
              for m in range(EC)]
        KT = [persist.tile([P, T_KV], mm_dt, name=f"kt_sb{m}", tag=f"kt{m}")
              for m in range(EC)]
        V3 = [persist.tile([P, H, 66], mm_dt, name=f"v3_sb{t}", tag=f"v3{t}")
              for t in range(n_kt)]
        ORT = [persist.tile([P, T_Q], mm_dt, name=f"ort_sb{m}", tag=f"ort{m}")
               for m in range(EC)]
        # sumexp rows live at partition 64 (even head) and 32 (odd head):
        # SBUF accesses may only start at partitions {0, 32, 64, 96} and
        # matmul operand base partitions only at {0, 32, 64}.
        SE = [persist.tile([65, T_Q], F32, name=f"se_sb{hp}", tag=f"se{hp}")
              for hp in range(H // 2)]
        # all 8 sumexp rows gathered here so one DVE reciprocal covers them
        # (single-partition reciprocals cost ~8us each on HW)
        G = persist.tile([H, T_Q], F32, name="g_se")
        ones1 = persist.tile([1, P], F32, name="ones1")
        sel2 = persist.tile([65, P], F32, name="sel2")
        bv_row = persist.tile([1, E], F32, name="bv_row")
        bo_row = persist.tile([1, E], F32, name="bo_row")

        nc.vector.memset(ones1, 1.0)
        # sel2 rows 64 / 32 are 64-wide ones rows used as K=1 lhsT to
        # broadcast the even / odd head's reciprocal sumexp across 64
        # output partitions.
        nc.vector.memset(sel2[64:65, 0:64], 1.0)
        nc.vector.memset(sel2[32:33, 0:64], 1.0)
        # bf16 strided writes (ACT out / gpsimd memset with a gappy AP)
        # wedge the device — memset the whole tile (contiguous) instead;
        # the V gelu lands via a contiguous staging tile + strided DMA.
        for t in range(n_kt):
            nc.gpsimd.memset(V3[t], 1.0)
        for k in range(EC):
            nc.sync.dma_start(out=wo_sb[k], in_=io["wo_t"][k * P:(k + 1) * P, :])
        nc.sync.dma_start(out=bv_row, in_=io["bv_row"])
        nc.sync.dma_start(out=bo_row, in_=io["bo_row"])

        # ---------------- phase A: projections ----------------
        with tc.tile_pool(name="projw", bufs=1) as projw, \
             tc.tile_pool(name="xtp", bufs=2) as xtp:
            wq_sb = [projw.tile([P, E], mm_dt, name=f"wq_sb{k}", tag=f"wq{k}")
                     for k in range(EC)]
            wk_sb = [projw.tile([P, E], mm_dt, name=f"wk_sb{k}", tag=f"wk{k}")
                     for k in range(EC)]
            wv_sb = [projw.tile([P, E], mm_dt, name=f"wv_sb{k}", tag=f"wv{k}")
                     for k in range(EC)]
            bq_sb = [projw.tile([P, 1], F32, name=f"bq_sb{m}", tag=f"bq{m}")
                     for m in range(EC)]
            bk_sb = [projw.tile([P, 1], F32, name=f"bk_sb{m}", tag=f"bk{m}")
                     for m in range(EC)]
            for k in range(EC):
                ksl = slice(k * P, (k + 1) * P)
                nc.sync.dma_start(out=wq_sb[k], in_=io["wq_t"][ksl, :])
                nc.sync.dma_start(out=wk_sb[k], in_=io["wk_t"][ksl, :])
                nc.sync.dma_start(out=wv_sb[k], in_=io["wv_t"][ksl, :])
                nc.sync.dma_start(out=bq_sb[k], in_=io["bq_col"][ksl, :])
                nc.sync.dma_start(out=bk_sb[k], in_=io["bk_col"][ksl, :])

            # 1024-token group pairs: one [128, 1024] PSUM + one gelu per
            # (m, pair) for K^T / Q^T.
            for gp in range(n_g // 2 if lvl >= 1 else 0):
                g0, g1 = 2 * gp, 2 * gp + 1
                psl = slice(gp * 1024, (gp + 1) * 1024)
                xt = {}
                for (gi, g) in ((0, g0), (1, g1)):
                    for k in range(EC):
                        xt[gi, k] = xtp.tile([P, 512], mm_dt,
                                             name=f"xt_g{g}k{k}",
                                             tag=f"xt{gi}_{k}")
                        nc.sync.dma_start(
                            out=xt[gi, k],
                            in_=xkv[k * P:(k + 1) * P, g * 512:(g + 1) * 512])
                # K^T chunks
                for m in range(EC):
                    msl = slice(m * P, (m + 1) * P)
                    ps = psp.tile([P, 1024], F32, name=f"ps_k{gp}_{m}",
                                  tag="st", bufs=3)
                    for gi in range(2):
                        for k in range(EC):
                            nc.tensor.matmul(ps[:, gi * 512:(gi + 1) * 512],
                                             lhsT=wk_sb[k][:, msl],
                                             rhs=xt[gi, k],
                                             start=(k == 0), stop=(k == EC - 1))
                    nc.scalar.activation(KT[m][:, psl], ps, AF.Gelu,
                                         bias=bk_sb[m])
                # Q^T chunks (only the core's own tokens)
                if (g1 + 1) * 512 <= T_Q:
                    for m in range(EC):
                        msl = slice(m * P, (m + 1) * P)
                        ps = psp.tile([P, 1024], F32, name=f"ps_q{gp}_{m}",
                                      tag="st", bufs=3)
                        for gi in range(2):
                            for k in range(EC):
                                nc.tensor.matmul(
                                    ps[:, gi * 512:(gi + 1) * 512],
                                    lhsT=wq_sb[k][:, msl], rhs=xt[gi, k],
                                    start=(k == 0), stop=(k == EC - 1))
                        nc.scalar.activation(QT[m][:, psl], ps, AF.Gelu,
                                             bias=bq_sb[m])
                # V natural [token, feature], bias seeded via K=1 fp32 matmul
                for s8 in range(8):
                    t = gp * 8 + s8
                    gi, s4 = divmod(s8, 4)
                    ps = psp.tile([P, E], F32, name=f"ps_v{t}", tag="st",
                                  bufs=3)
                    nc.tensor.matmul(ps, lhsT=ones1, rhs=bv_row,
                                     start=True, stop=False)
                    for k in range(EC):
                        nc.tensor.matmul(ps,
                                         lhsT=xt[gi, k][:, s4 * P:(s4 + 1) * P],
                                         rhs=wv_sb[k],
                                         start=False, stop=(k == EC - 1))
                    vst = xtp.tile([P, E], mm_dt, name=f"vst{t}", tag="vst",
                                   bufs=3)
                    nc.scalar.activation(vst, ps, AF.Gelu)
                    nc.sync.dma_start(
                        out=V3[t][:, :, 0:64],
                        in_=vst.rearrange("p (h d) -> p h d", h=H))

        # ---------------- phase B: attention ----------------
        with tc.tile_pool(name="ep", bufs=4) as ep, \
             tc.tile_pool(name="tmpp", bufs=2) as tmpp, \
             tc.tile_pool(name="outp", bufs=3) as outp:
            for hp in range(H // 2 if lvl >= 2 else 0):
                he, ho = 2 * hp, 2 * hp + 1
                for q in range(n_qg):
                    qsl = slice(q * 512, (q + 1) * 512)
                    pv0 = psp.tile([65, 512], F32, name=f"pv0_{hp}_{q}",
                                   tag="pv", bufs=2)
                    pv1 = psp.tile([65, 512], F32, name=f"pv1_{hp}_{q}",
                                   tag="pv", bufs=2)
                    for jp in range(n_kt // 2):
                        st0 = psp.tile([P, 1024], F32,
                                       name=f"st0_{hp}{q}{jp}",
                                       tag="st", bufs=3)
                        st1 = psp.tile([P, 1024], F32,
                                       name=f"st1_{hp}{q}{jp}",
                                       tag="st", bufs=3)
                        for u in range(2):
                            kt = jp * 2 + u
                            ksl = slice(kt * P, (kt + 1) * P)
                            usl = slice(u * 512, (u + 1) * 512)
                            nc.tensor.matmul(st0[:, usl],
                                             lhsT=KT[hp][0:64, ksl],
                                             rhs=QT[hp][0:64, qsl],
                                             start=True, stop=True)
                            nc.tensor.matmul(st1[:, usl],
                                             lhsT=KT[hp][64:128, ksl],
                                             rhs=QT[hp][64:128, qsl],
                                             start=True, stop=True)
                        e0 = ep.tile([P, 1024], mm_dt, name=f"e0_{hp}{q}{jp}",
                                     tag="e0")
                        e1 = ep.tile([P, 1024], mm_dt, name=f"e1_{hp}{q}{jp}",
                                     tag="e1")
                        nc.scalar.activation(e0, st0, AF.Exp, scale=SCALE)
                        nc.scalar.activation(e1, st1, AF.Exp, scale=SCALE)
                        if lvl < 3:
                            continue
                        for u in range(2):
                            kt = jp * 2 + u
                            usl = slice(u * 512, (u + 1) * 512)
                            nc.tensor.matmul(pv0, lhsT=V3[kt][:, he, 0:65],
                                             rhs=e0[:, usl],
                                             start=(kt == 0),
                                             stop=(kt == n_kt - 1))
                            nc.tensor.matmul(pv1, lhsT=V3[kt][:, ho, 0:65],
                                             rhs=e1[:, usl],
                                             start=(kt == 0),
                                             stop=(kt == n_kt - 1))
                    if lvl < 3:
                        continue
                    # unit drain: even head rows are partition-aligned, the
                    # odd head hops across partitions via SBUF + DMA; sumexp
                    # rows go via tmp tiles into the gather tile G.
                    nc.vector.tensor_copy(ORT[hp][0:64, qsl], pv0[0:64, :])
                    tmp_v = tmpp.tile([64, 512], mm_dt, name=f"tv_{hp}_{q}",
                                      tag="tv")
                    ts0 = tmpp.tile([65, 512], F32, name=f"ts0_{hp}_{q}",
                                    tag="ts0")
                    ts1 = tmpp.tile([65, 512], F32, name=f"ts1_{hp}_{q}",
                                    tag="ts1")
                    nc.vector.tensor_copy(tmp_v, pv1[0:64, :])
                    nc.vector.tensor_copy(ts0[64:65, :], pv0[64:65, :])
                    nc.vector.tensor_copy(ts1[64:65, :], pv1[64:65, :])
                    nc.sync.dma_start(out=ORT[hp][64:128, qsl], in_=tmp_v)
                    nc.sync.dma_start(out=G[he:he + 1, qsl],
                                      in_=ts0[64:65, :])
                    nc.sync.dma_start(out=G[ho:ho + 1, qsl],
                                      in_=ts1[64:65, :])

            # ---------------- normalization ----------------
            if lvl >= 4:
                nc.vector.reciprocal(G, G)
                for hp in range(H // 2):
                    nc.sync.dma_start(out=SE[hp][64:65, :],
                                      in_=G[2 * hp:2 * hp + 1, :])
                    nc.sync.dma_start(out=SE[hp][32:33, :],
                                      in_=G[2 * hp + 1:2 * hp + 2, :])
            for hp in range(H // 2 if lvl >= 4 else 0):
                R = psp.tile([P, T_Q], F32, name=f"R_{hp}", tag="st", bufs=3)
                for q in range(n_qg):
                    qsl = slice(q * 512, (q + 1) * 512)
                    nc.tensor.matmul(R[0:64, qsl], lhsT=sel2[64:65, 0:64],
                                     rhs=SE[hp][64:65, qsl],
                                     start=True, stop=True)
                    nc.tensor.matmul(R[64:128, qsl], lhsT=sel2[32:33, 0:64],
                                     rhs=SE[hp][32:33, qsl],
                                     start=True, stop=True)
                nc.vector.tensor_mul(ORT[hp], ORT[hp], R)

            # ---------------- phase C: output projection ----------------
            for t in range(T_Q // P if lvl >= 5 else 0):
                tsl = slice(t * P, (t + 1) * P)
                ps = psp.tile([P, E], F32, name=f"ps_o{t}", tag="st", bufs=3)
                nc.tensor.matmul(ps, lhsT=ones1, rhs=bo_row,
                                 start=True, stop=False)
                for m in range(EC):
                    nc.tensor.matmul(ps, lhsT=ORT[m][:, tsl], rhs=wo_sb[m],
                                     start=False, stop=(m == EC - 1))
                ot = outp.tile([P, E], F32, name=f"ot_{t}", tag="ot")
                nc.scalar.activation(ot, ps, AF.Gelu)
                nc.sync.dma_start(out=out[tsl, :], in_=ot)


def make_nc(T_KV, T_Q, num_devices=N_CORES, mm_dt=BF16, debug=False,
            stop_after="all"):
    nc = bacc.Bacc("TRN2", target_bir_lowering=False, debug=debug,
                   num_devices=num_devices)
    io = {
        "xkv_t": nc.dram_tensor("xkv_t", [E, T_KV], mm_dt,
                                kind="ExternalInput").ap(),
        "wq_t": nc.dram_tensor("wq_t", [E, E], mm_dt,
                               kind="ExternalInput").ap(),
        "wk_t": nc.dram_tensor("wk_t", [E, E], mm_dt,
                               kind="ExternalInput").ap(),
        "wv_t": nc.dram_tensor("wv_t", [E, E], mm_dt,
                               kind="ExternalInput").ap(),
        "wo_t": nc.dram_tensor("wo_t", [E, E], mm_dt,
                               kind="ExternalInput").ap(),
        "bq_col": nc.dram_tensor("bq_col", [E, 1], F32,
                                 kind="ExternalInput").ap(),
        "bk_col": nc.dram_tensor("bk_col", [E, 1], F32,
                                 kind="ExternalInput").ap(),
        "bv_row": nc.dram_tensor("bv_row", [1, E], F32,
                                 kind="ExternalInput").ap(),
        "bo_row": nc.dram_tensor("bo_row", [1, E], F32,
                                 kind="ExternalInput").ap(),
        "out": nc.dram_tensor("out", [T_Q, E], F32, kind="ExternalOutput").ap(),
    }
    with tile.TileContext(nc) as tc:
        build(tc, io, T_KV=T_KV, T_Q=T_Q, mm_dt=mm_dt, stop_after=stop_after)
    nc.compile()
    return nc


def make_in_maps(x, Wq, bq, Wk, bk, Wv, bv, Wo, bo, mm_np=None):
    if mm_np is None:
        import ml_dtypes
        mm_np = ml_dtypes.bfloat16
    castm = lambda a: np.ascontiguousarray(np.asarray(a).astype(mm_np))
    castf = lambda a: np.ascontiguousarray(np.asarray(a, dtype=np.float32))
    base = {
        "wq_t": castm(np.asarray(Wq).T),
        "wk_t": castm(np.asarray(Wk).T),
        "wv_t": castm(np.asarray(Wv).T),
        "wo_t": castm(np.asarray(Wo).T),
        "bq_col": castf(np.asarray(bq)[:, None]),
        "bk_col": castf(np.asarray(bk)[:, None]),
        "bv_row": castf(np.asarray(bv)[None, :]),
        "bo_row": castf(np.asarray(bo)[None, :]),
    }
    x = np.asarray(x)
    half_len = S // 2
    in_maps = []
    for c in range(N_CORES):
        b, half = divmod(c, 2)
        xb = x[b]
        mine = xb[half * half_len:(half + 1) * half_len]
        oth = xb[(1 - half) * half_len:(2 - half) * half_len]
        m = dict(base)
        m["xkv_t"] = castm(np.concatenate([mine, oth], axis=0).T)
        in_maps.append(m)
    return in_maps


_NC_CACHE = {}


def _get_full_nc():
    if "full" not in _NC_CACHE:
        _NC_CACHE["full"] = make_nc(T_KV=S, T_Q=S // 2)
    return _NC_CACHE["full"]


def run_on_hw(in_maps, trace=False, **kw):
    from concourse.bass_utils import run_bass_kernel_spmd
    nc = _get_full_nc()
    return run_bass_kernel_spmd(nc, in_maps, core_ids=list(range(N_CORES)),
                                trace=trace, **kw)


def kernel(x, Wq, bq, Wk, bk, Wv, bv, Wo, bo):
    in_maps = make_in_maps(x, Wq, bq, Wk, bk, Wv, bv, Wo, bo)
    res = run_on_hw(in_maps)
    half_len = S // 2
    out = np.empty((B, S, E), np.float32)
    for c in range(N_CORES):
        b, half = divmod(c, 2)
        out[b, half * half_len:(half + 1) * half_len, :] = \
            res.results[c]["out"]
    return out


# revision 22
# speedup vs baseline: 2.8242x; 1.0192x over previous
"""Trainium2 Bass kernel for nn_AttentionBlock (B=4, S=2048, E=512, H=8).

Sharding (no cross-core communication):
  core c owns batch b = c//2 and output rows for tokens half = c%2
  (1024 tokens).  Each core computes Q for its own 1024 tokens and K/V for
  the full batch (2048 tokens); K/V projections are duplicated between the
  two cores of a batch (cheap) so attention and the output projection are
  fully local.

Host-side layout prep (inside kernel(), numpy only):
  - x[b] is permuted so the core's own 1024 query tokens are rows 0:1024,
    then transposed to feature-major xkv_t [512, 2048].  Softmax/PV are
    permutation-invariant in the key axis, so K/V token order is irrelevant.
  - weights are passed as W.T [E, E] so lhsT/rhs tiles are direct DMA loads.
  - x and weights are cast to bf16 (fp32 matmuls lower to two PE passes on
    TRN2 — half throughput); biases, PSUM accumulation, softmax statistics
    and the normalization stay fp32.

Device dataflow (per core):
  A) QT[f, t] = gelu(Wq @ x^T + bq) via PE (lhsT = Wq.T chunk, rhs = x^T),
     same for KT; V natural [t, f] via (lhsT = x^T chunk, rhs = Wv.T chunk)
     with the bias row pre-seeded into PSUM by a K=1 fp32 ones matmul.
  B) per head pair (even head at partitions 0:64, odd at 64:128 so the
     K=64 S^T matmuls auto-pack into disjoint PE row groups):
     S^T tile = K^T_h(ktile) @ Q^T_h -> PSUM (fp32), one ACT Exp(scale=1/8)
     per [128,1024] tile straight from PSUM into bf16 E tiles, then PV
     accumulates O^T_unnorm and sumexp in one matmul via a [V | 1] lhsT.
  C) per-head normalization deferred to one pass: sumexp rows at SBUF
     partitions 64 (even) / 32 (odd), fast-approx reciprocal on DVE,
     broadcast across 64 output partitions with single-instruction K=1
     selector matmuls (even -> R rows 0:64, odd -> R rows 64:128 — these
     must stay separate accumulation groups: mixing lhsT base partitions
     inside one start/stop group crashes the HW), one DVE multiply per
     ORT chunk.
  D) out = gelu(ORT chunks @ Wo.T + bo) with bo PSUM-seeded; DMA out.
"""

import numpy as np

import concourse.bass as bass
import concourse.tile as tile
from concourse import bacc, mybir

F32 = mybir.dt.float32
BF16 = mybir.dt.bfloat16
AF = mybir.ActivationFunctionType

E = 512          # embed dim
H = 8            # heads
D = 64           # head dim
P = 128          # partitions
EC = E // P      # 128-row chunks of the embed dim
B = 4
S = 2048
N_CORES = 8
SCALE = 0.125    # 1/sqrt(D)


def build(tc, io, T_KV, T_Q, mm_dt=BF16, stop_after="all"):
    """Emit the per-core program.  T_KV: key/value tokens; T_Q: query tokens
    (the first T_Q columns of xkv_t).  mm_dt: dtype of matmul operands."""
    nc = tc.nc
    stages = ["setup", "proj", "st", "pv", "norm", "all"]
    lvl = stages.index(stop_after)
    n_g = T_KV // 512        # 512-token groups for projections
    n_qg = T_Q // 512        # q groups (S^T rhs is N=512)
    n_kt = T_KV // P         # 128-token key tiles
    assert T_KV % 1024 == 0 and T_Q % 1024 == 0

    xkv = io["xkv_t"]
    out = io["out"]

    with tc.tile_pool(name="persist", bufs=1) as persist, \
         tc.tile_pool(name="ps", space="PSUM", bufs=1) as psp:
        # ---------------- persistent SBUF state ----------------
        wo_sb = [persist.tile([P, E], mm_dt, name=f"wo_sb{k}", tag=f"wo{k}")
                 for k in range(EC)]
        QT = [persist.tile([P, T_Q], mm_dt, name=f"qt_sb{m}", tag=f"qt{m}")
              for m in range(EC)]
        KT = [persist.tile([P, T_KV], mm_dt, name=f"kt_sb{m}", tag=f"kt{m}")
              for m in range(EC)]
        V3 = [persist.tile([P, H, 66], mm_dt, name=f"v3_sb{t}", tag=f"v3{t}")
              for t in range(n_kt)]
        ORT = [persist.tile([P, T_Q], mm_dt, name=f"ort_sb{m}", tag=f"ort{m}")
               for m in range(EC)]
        # sumexp rows live at partition 64 (even head) and 32 (odd head):
        # SBUF accesses may only start at partitions {0, 32, 64, 96} and
        # matmul operand base partitions only at {0, 32, 64}.
        SE = [persist.tile([65, T_Q], F32, name=f"se_sb{hp}", tag=f"se{hp}")
              for hp in range(H // 2)]
        # all 8 sumexp rows gathered here so one DVE reciprocal covers them
        # (single-partition reciprocals cost ~8us each on HW)
        G = persist.tile([H, T_Q], F32, name="g_se")
        sel2 = persist.tile([65, P], F32, name="sel2")
        # bias rows broadcast across all 128 partitions (partition-stride-0
        # DMA) so biases land with one DVE add instead of seed matmuls
        bv_bc = persist.tile([P, E], F32, name="bv_bc")
        bo_bc = persist.tile([P, E], F32, name="bo_bc")

        # sel2 rows 64 / 32 are 64-wide ones rows used as K=1 lhsT to
        # broadcast the even / odd head's reciprocal sumexp across 64
        # output partitions.
        nc.vector.memset(sel2[64:65, 0:64], 1.0)
        nc.vector.memset(sel2[32:33, 0:64], 1.0)
        # bf16 strided writes (ACT out / gpsimd memset with a gappy AP)
        # wedge the device — memset the whole tile (contiguous) instead;
        # the V gelu lands via a contiguous staging tile + strided DMA.
        for t in range(n_kt):
            nc.gpsimd.memset(V3[t], 1.0)

        # ---------------- phase A: projections ----------------
        with tc.tile_pool(name="projw", bufs=1) as projw, \
             tc.tile_pool(name="xtp", bufs=2) as xtp:
            wq_sb = [projw.tile([P, E], mm_dt, name=f"wq_sb{k}", tag=f"wq{k}")
                     for k in range(EC)]
            wk_sb = [projw.tile([P, E], mm_dt, name=f"wk_sb{k}", tag=f"wk{k}")
                     for k in range(EC)]
            wv_sb = [projw.tile([P, E], mm_dt, name=f"wv_sb{k}", tag=f"wv{k}")
                     for k in range(EC)]
            bq_sb = [projw.tile([P, 1], F32, name=f"bq_sb{m}", tag=f"bq{m}")
                     for m in range(EC)]
            bk_sb = [projw.tile([P, 1], F32, name=f"bk_sb{m}", tag=f"bk{m}")
                     for m in range(EC)]
            for k in range(EC):
                ksl = slice(k * P, (k + 1) * P)
                nc.sync.dma_start(out=wq_sb[k], in_=io["wq_t"][ksl, :])
                nc.sync.dma_start(out=wk_sb[k], in_=io["wk_t"][ksl, :])
                nc.sync.dma_start(out=wv_sb[k], in_=io["wv_t"][ksl, :])
                nc.sync.dma_start(out=bq_sb[k], in_=io["bq_col"][ksl, :])
                nc.sync.dma_start(out=bk_sb[k], in_=io["bk_col"][ksl, :])
            # non-critical loads after the projection-path DMAs
            for k in range(EC):
                nc.sync.dma_start(out=wo_sb[k],
                                  in_=io["wo_t"][k * P:(k + 1) * P, :])
            for (dst, srcap) in ((bv_bc, io["bv_row"]), (bo_bc, io["bo_row"])):
                bc = bass.AP(tensor=srcap.tensor, offset=srcap.offset,
                             ap=[[0, P]] + list(srcap.ap[1:]))
                nc.sync.dma_start(out=dst, in_=bc)

            # 1024-token group pairs: one [128, 1024] PSUM + one gelu per
            # (m, pair) for K^T / Q^T.
            for gp in range(n_g // 2 if lvl >= 1 else 0):
                g0, g1 = 2 * gp, 2 * gp + 1
                psl = slice(gp * 1024, (gp + 1) * 1024)
                xt = {}
                for (gi, g) in ((0, g0), (1, g1)):
                    for k in range(EC):
                        xt[gi, k] = xtp.tile([P, 512], mm_dt,
                                             name=f"xt_g{g}k{k}",
                                             tag=f"xt{gi}_{k}")
                        nc.sync.dma_start(
                            out=xt[gi, k],
                            in_=xkv[k * P:(k + 1) * P, g * 512:(g + 1) * 512])
                # K^T chunks
                for m in range(EC):
                    msl = slice(m * P, (m + 1) * P)
                    ps = psp.tile([P, 1024], F32, name=f"ps_k{gp}_{m}",
                                  tag="st", bufs=3)
                    for gi in range(2):
                        for k in range(EC):
                            nc.tensor.matmul(ps[:, gi * 512:(gi + 1) * 512],
                                             lhsT=wk_sb[k][:, msl],
                                             rhs=xt[gi, k],
                                             start=(k == 0), stop=(k == EC - 1))
                    nc.scalar.activation(KT[m][:, psl], ps, AF.Gelu,
                                         bias=bk_sb[m])
                # Q^T chunks (only the core's own tokens)
                if (g1 + 1) * 512 <= T_Q:
                    for m in range(EC):
                        msl = slice(m * P, (m + 1) * P)
                        ps = psp.tile([P, 1024], F32, name=f"ps_q{gp}_{m}",
                                      tag="st", bufs=3)
                        for gi in range(2):
                            for k in range(EC):
                                nc.tensor.matmul(
                                    ps[:, gi * 512:(gi + 1) * 512],
                                    lhsT=wq_sb[k][:, msl], rhs=xt[gi, k],
                                    start=(k == 0), stop=(k == EC - 1))
                        nc.scalar.activation(QT[m][:, psl], ps, AF.Gelu,
                                             bias=bq_sb[m])
                # V natural [token, feature], bias seeded via K=1 fp32 matmul
                for s8 in range(8):
                    t = gp * 8 + s8
                    gi, s4 = divmod(s8, 4)
                    ps = psp.tile([P, E], F32, name=f"ps_v{t}", tag="st",
                                  bufs=3)
                    for k in range(EC):
                        nc.tensor.matmul(ps,
                                         lhsT=xt[gi, k][:, s4 * P:(s4 + 1) * P],
                                         rhs=wv_sb[k],
                                         start=(k == 0), stop=(k == EC - 1))
                    nc.vector.tensor_add(ps, ps, bv_bc)
                    vst = xtp.tile([P, E], mm_dt, name=f"vst{t}", tag="vst",
                                   bufs=3)
                    nc.scalar.activation(vst, ps, AF.Gelu)
                    nc.sync.dma_start(
                        out=V3[t][:, :, 0:64],
                        in_=vst.rearrange("p (h d) -> p h d", h=H))

        # ---------------- phase B: attention ----------------
        # q-half outer so each half's normalization + output projection
        # overlaps the other half's attention (shrinks the serial tail).
        with tc.tile_pool(name="ep", bufs=4) as ep, \
             tc.tile_pool(name="tmpp", bufs=2) as tmpp, \
             tc.tile_pool(name="outp", bufs=3) as outp:
            for q in range(n_qg if lvl >= 2 else 0):
                qsl = slice(q * 512, (q + 1) * 512)
                for hp in range(H // 2):
                    he, ho = 2 * hp, 2 * hp + 1
                    pv0 = psp.tile([65, 512], F32, name=f"pv0_{hp}_{q}",
                                   tag="pv", bufs=2)
                    pv1 = psp.tile([65, 512], F32, name=f"pv1_{hp}_{q}",
                                   tag="pv", bufs=2)
                    for jp in range(n_kt // 2):
                        st0 = psp.tile([P, 1024], F32,
                                       name=f"st0_{hp}{q}{jp}",
                                       tag="st", bufs=3)
                        st1 = psp.tile([P, 1024], F32,
                                       name=f"st1_{hp}{q}{jp}",
                                       tag="st", bufs=3)
                        for u in range(2):
                            kt = jp * 2 + u
                            ksl = slice(kt * P, (kt + 1) * P)
                            usl = slice(u * 512, (u + 1) * 512)
                            nc.tensor.matmul(st0[:, usl],
                                             lhsT=KT[hp][0:64, ksl],
                                             rhs=QT[hp][0:64, qsl],
                                             start=True, stop=True)
                            nc.tensor.matmul(st1[:, usl],
                                             lhsT=KT[hp][64:128, ksl],
                                             rhs=QT[hp][64:128, qsl],
                                             start=True, stop=True)
                        e0 = ep.tile([P, 1024], mm_dt, name=f"e0_{hp}{q}{jp}",
                                     tag="e0")
                        e1 = ep.tile([P, 1024], mm_dt, name=f"e1_{hp}{q}{jp}",
                                     tag="e1")
                        nc.scalar.activation(e0, st0, AF.Exp, scale=SCALE)
                        nc.scalar.activation(e1, st1, AF.Exp, scale=SCALE)
                        if lvl < 3:
                            continue
                        for u in range(2):
                            kt = jp * 2 + u
                            usl = slice(u * 512, (u + 1) * 512)
                            nc.tensor.matmul(pv0, lhsT=V3[kt][:, he, 0:65],
                                             rhs=e0[:, usl],
                                             start=(kt == 0),
                                             stop=(kt == n_kt - 1))
                            nc.tensor.matmul(pv1, lhsT=V3[kt][:, ho, 0:65],
                                             rhs=e1[:, usl],
                                             start=(kt == 0),
                                             stop=(kt == n_kt - 1))
                    if lvl < 3:
                        continue
                    # unit drain: even head rows are partition-aligned, the
                    # odd head hops across partitions via SBUF + DMA; sumexp
                    # rows go via tmp tiles into the gather tile G.
                    nc.vector.tensor_copy(ORT[hp][0:64, qsl], pv0[0:64, :])
                    tmp_v = tmpp.tile([64, 512], mm_dt, name=f"tv_{hp}_{q}",
                                      tag="tv")
                    ts0 = tmpp.tile([65, 512], F32, name=f"ts0_{hp}_{q}",
                                    tag="ts0")
                    ts1 = tmpp.tile([65, 512], F32, name=f"ts1_{hp}_{q}",
                                    tag="ts1")
                    nc.vector.tensor_copy(tmp_v, pv1[0:64, :])
                    nc.vector.tensor_copy(ts0[64:65, :], pv0[64:65, :])
                    nc.vector.tensor_copy(ts1[64:65, :], pv1[64:65, :])
                    nc.sync.dma_start(out=ORT[hp][64:128, qsl], in_=tmp_v)
                    nc.sync.dma_start(out=G[he:he + 1, qsl],
                                      in_=ts0[64:65, :])
                    nc.sync.dma_start(out=G[ho:ho + 1, qsl],
                                      in_=ts1[64:65, :])

            # ---------------- normalization ----------------
            if lvl >= 4:
                nc.vector.reciprocal(G, G)
                for hp in range(H // 2):
                    nc.sync.dma_start(out=SE[hp][64:65, :],
                                      in_=G[2 * hp:2 * hp + 1, :])
                    nc.sync.dma_start(out=SE[hp][32:33, :],
                                      in_=G[2 * hp + 1:2 * hp + 2, :])
                for hp in range(H // 2):
                    R = psp.tile([P, T_Q], F32, name=f"R_{hp}", tag="st",
                                 bufs=3)
                    for q2 in range(n_qg):
                        q2sl = slice(q2 * 512, (q2 + 1) * 512)
                        nc.tensor.matmul(R[0:64, q2sl],
                                         lhsT=sel2[64:65, 0:64],
                                         rhs=SE[hp][64:65, q2sl],
                                         start=True, stop=True)
                        nc.tensor.matmul(R[64:128, q2sl],
                                         lhsT=sel2[32:33, 0:64],
                                         rhs=SE[hp][32:33, q2sl],
                                         start=True, stop=True)
                    nc.vector.tensor_mul(ORT[hp], ORT[hp], R)

                # ---------------- per-half normalization ----------------
                if lvl < 4:
                    continue
                nc.vector.reciprocal(G[:, qsl], G[:, qsl])
                for hp in range(H // 2):
                    nc.sync.dma_start(out=SE[hp][64:65, qsl],
                                      in_=G[2 * hp:2 * hp + 1, qsl])
                    nc.sync.dma_start(out=SE[hp][32:33, qsl],
                                      in_=G[2 * hp + 1:2 * hp + 2, qsl])
                for hp in range(H // 2):
                    R = psp.tile([P, 512], F32, name=f"R_{hp}_{q}", tag="st",
                                 bufs=3)
                    nc.tensor.matmul(R[0:64, :], lhsT=sel2[64:65, 0:64],
                                     rhs=SE[hp][64:65, qsl],
                                     start=True, stop=True)
                    nc.tensor.matmul(R[64:128, :], lhsT=sel2[32:33, 0:64],
                                     rhs=SE[hp][32:33, qsl],
                                     start=True, stop=True)
                    nc.vector.tensor_mul(ORT[hp][:, qsl], ORT[hp][:, qsl], R)

            # ---------------- phase C (after all halves: keeps the Gelu
            # table loads from interleaving with Exp) ----------------
            if lvl >= 5:
                for t in range(T_Q // P):
                    tsl = slice(t * P, (t + 1) * P)
                    ps = psp.tile([P, E], F32, name=f"ps_o{t}", tag="st",
                                  bufs=3)
                    for m in range(EC):
                        nc.tensor.matmul(ps, lhsT=ORT[m][:, tsl],
                                         rhs=wo_sb[m],
                                         start=(m == 0), stop=(m == EC - 1))
                    nc.vector.tensor_add(ps, ps, bo_bc)
                    ot = outp.tile([P, E], F32, name=f"ot_{t}", tag="ot")
                    nc.scalar.activation(ot, ps, AF.Gelu)
                    nc.sync.dma_start(out=out[tsl, :], in_=ot)
